# revision 1
# baseline (speedup 1.0000x reference)
"""Clockwork RNN (CWRNN) Trainium2 Bass kernel.

Problem (hardcoded from spec): B=512, T=192, DX=32, DY=4 heads, DH1=256,
DH2=512, update rates (1,2,4) over hidden blocks of (128,64,64) units.

Sharding: 8 cores = 4 heads x 2 batch-halves (B_core=256). Heads are fully
independent; batch is data-parallel.

Per-core dataflow (everything transposed: units on partitions, batch free):
  - state h kept as SBUF tile [128, 2, 256]: h[p, c, b] = h_unit(c*128+p).
  - per step j: cand^T accumulated in PSUM via matmuls
      wxb^T @ [x_t; 1]  (K=33, bias folded in via a ones row)
    + w_h[128:256]^T @ h1  and  w_h[0:128]^T @ h0  (K=128 each),
    then tanh on the scalar engine writes updated units back into h.
    Clock gating = only computing/writing the first k units (partition rows),
    so the schedule's "where" is free.
  - matmul operands are float16: full-rate PE at any size, separate
    pulled-ahead weight loads, 10-bit mantissa (~7e-4 end-to-end error;
    all values here are small so fp16 range is safe).
  - final MLP: hid = relu(W1^T h + b1), y = W2 . hid (+ b2 on host).
"""

import contextlib

import numpy as np
import ml_dtypes

import concourse.bass as bass
import concourse.mybir as mybir
import concourse.tile as tile
from concourse import bacc
from concourse.bass_utils import run_bass_kernel_spmd

F32 = mybir.dt.float32
TANH = mybir.ActivationFunctionType.Tanh
RELU = mybir.ActivationFunctionType.Relu

B, T, DX, DY, DH1, DH2 = 512, 192, 32, 4, 256, 512
KX = DX + 1          # w_x rows + folded bias row
BC = B // 2          # batch per core (256)
TW = 16              # timesteps per x window DMA

# matmul operand dtype: "f16" (10-bit mantissa, ~7e-4 end-to-end err, 2-byte
# so weight loads are separate instructions the PE pulls ahead), "f32r"
# (11-bit mantissa but fused 4-byte weight loads serialize ~320ns/matmul),
# "bf16" (~5e-3 err), or "f32" (exact but quarter-rate PE)
MM_DT = "f16"
# batch chunks per core (2 shortens the serial chain but needs full-rate
# matmuls at moving dim 128, which f32r does not have)
NCH = {"f32r": 1, "bf16": 2, "f16": 1, "f32": 2}
# merge the type-3 m0+m1 tanh into one activation
MERGE3 = True
PS_BUFS = 2
PSM1_BUFS = 1

_nc_cache = {}


def _null():
    return contextlib.nullcontext()


def _mm_dtype(name):
    return {"f32r": mybir.dt.float32r, "bf16": mybir.dt.bfloat16,
            "f16": mybir.dt.float16, "f32": mybir.dt.float32}[name]


def _round_f32r(a):
    v = np.ascontiguousarray(a, np.float32).view(np.uint32)
    r = ((v + np.uint32(0x800)) & np.uint32(0xFFFFF000))
    return r.view(np.float32).copy()


def _host_cast(a, dt_name):
    if dt_name == "f32r":
        return _round_f32r(a)
    if dt_name == "bf16":
        return np.asarray(a, np.float32).astype(ml_dtypes.bfloat16)
    if dt_name == "f16":
        return np.asarray(a, np.float32).astype(np.float16)
    return np.ascontiguousarray(a, np.float32)


def _step_type(j):
    if (j + 1) % 4 == 0:
        return 3
    if (j + 1) % 2 == 0:
        return 2
    return 1


def _feat_mask():
    feat = np.zeros((T, DX), np.float32)
    for j in range(T):
        n = {1: 16, 2: 24, 3: 32}[_step_type(j)]
        feat[j, :n] = 1.0
    return feat


def build_nc(mm_dt=MM_DT, nch=None, merge3=MERGE3, reps=1):
    DT = _mm_dtype(mm_dt)
    if nch is None:
        nch = NCH[mm_dt]
    cb = BC // nch
    nc = bacc.Bacc("TRN2", target_bir_lowering=False, debug=False)
    xt_d = nc.dram_tensor("xt", [KX, T, BC], DT, kind="ExternalInput")
    wh_d = nc.dram_tensor("wh", [DH1, DH1], DT, kind="ExternalInput")
    wxb_d = nc.dram_tensor("wxb", [KX, DH1], DT, kind="ExternalInput")
    w1_d = nc.dram_tensor("w1", [DH1, DH2], DT, kind="ExternalInput")
    b1_d = nc.dram_tensor("b1t", [128, 4], F32, kind="ExternalInput")
    w2_d = nc.dram_tensor("w2t", [128, 4], DT, kind="ExternalInput")
    y_d = nc.dram_tensor("y", [1, BC], F32, kind="ExternalOutput")

    with tile.TileContext(nc) as tc:
        with (
            tc.tile_pool(name="const", bufs=1) as cpool,
            tc.tile_pool(name="state", bufs=1) as spool,
            tc.tile_pool(name="xw", bufs=3) as xpool,
            tc.tile_pool(name="ps", bufs=PS_BUFS, space=bass.MemorySpace.PSUM) as pspool,
            tc.tile_pool(name="psm1", bufs=PSM1_BUFS, space=bass.MemorySpace.PSUM) as psm1pool,
            tc.tile_pool(name="ps2", bufs=1, space=bass.MemorySpace.PSUM) as ps2pool,
            tc.tile_pool(name="misc", bufs=1) as mpool,
        ):
            wh_sb = cpool.tile([128, 2, DH1], DT, tag="wh")
            for c in range(2):
                nc.sync.dma_start(wh_sb[:, c, :], wh_d[c * 128:(c + 1) * 128, :])
            wxb_sb = cpool.tile([KX, DH1], DT, tag="wxb")
            nc.sync.dma_start(wxb_sb[:], wxb_d[:])
            w1_sb = cpool.tile([128, 2, DH2], DT, tag="w1")
            for c in range(2):
                nc.sync.dma_start(w1_sb[:, c, :], w1_d[c * 128:(c + 1) * 128, :])
            b1_sb = cpool.tile([128, 4], F32, tag="b1")
            nc.sync.dma_start(b1_sb[:], b1_d[:])
            w2_sb = cpool.tile([128, 4], DT, tag="w2")
            nc.sync.dma_start(w2_sb[:], w2_d[:])

            # h is never zero-initialized: matmuls reading a still-unwritten
            # block of h are skipped (j=0,1) or K-narrowed (j=2,3), which is
            # the same math since those units are zero.
            h = spool.tile([128, 2, BC], DT, tag="h")

            # reps>1 wraps the whole network in a hardware loop for
            # timing measurements (wall-clock delta per iteration)
            with tc.For_i(0, reps, 1) if reps > 1 else _null():
                xw = None
                for j in range(T):
                    r = j % TW
                    if r == 0:
                        xw = xpool.tile([KX, TW, BC], DT, tag="xw")
                        nc.sync.dma_start(xw[:], xt_d[:, j:j + TW, :])
                    typ = _step_type(j)
                    m1p = {1: 0, 2: 64, 3: 128}[typ]

                    def emit_group(mms):
                        # one accumulation group per PSUM bank; the tanh
                        # reading a bank depends on that bank's stop matmul,
                        # so ACT never reads while PE writes the same bank
                        for i, (o, lt, rh) in enumerate(mms):
                            nc.tensor.matmul(o, lt, rh, start=(i == 0),
                                             stop=(i == len(mms) - 1))

                    acts = []
                    for ch in range(nch):
                        cs = slice(ch * cb, (ch + 1) * cb)

                        def c1(cols):
                            # h units 192:256 are first written at j=3; before
                            # that, narrow the h1 contraction to K=64 (the
                            # rest would multiply uninitialized zeros)
                            if j <= 3:
                                return (wh_sb[0:64, 1, cols], h[0:64, 1, cs])
                            return (wh_sb[:, 1, cols], h[:, 1, cs])

                        if typ == 3 and merge3:
                            # full-update steps: m0+m1 share one bank and one
                            # merged tanh (halves the ACT call count here)
                            psm = pspool.tile([128, 2, cb], F32, tag=f"ps{ch}")
                            mm = [(psm[:, 0, :], wxb_sb[:, 0:128], xw[:, r, cs]),
                                  (psm[:, 1, :], wxb_sb[:, 128:256], xw[:, r, cs])]
                            if j >= 2:
                                mm.append((psm[:, 0, :], *c1(slice(0, 128))))
                                mm.append((psm[:, 1, :], *c1(slice(128, 256))))
                            if j >= 1:
                                mm.append((psm[:, 0, :], wh_sb[:, 0, 0:128], h[:, 0, cs]))
                                mm.append((psm[:, 1, :], wh_sb[:, 0, 128:256], h[:, 0, cs]))
                            emit_group(mm)
                            acts.append((h[:, 0:2, cs], psm[:, 0:2, :]))
                            continue

                        ps = pspool.tile([128, cb], F32, tag=f"ps{ch}")
                        # m1 (units 128:128+m1p) in its own bank, emitted and
                        # activated FIRST: its tanh output h1 is the first
                        # thing the next (even) step's c1 matmuls need, while
                        # h0 is consumed a matmul later. m0 second.
                        if m1p:
                            pm1 = psm1pool.tile([m1p, cb], F32, tag=f"m1_{ch}")
                            me = slice(128, 128 + m1p)
                            mm1 = [(pm1[:], wxb_sb[:, me], xw[:, r, cs])]
                            if j >= 2:
                                mm1.append((pm1[:], *c1(me)))
                            if j >= 1:
                                mm1.append((pm1[:], wh_sb[:, 0, me], h[:, 0, cs]))
                            emit_group(mm1)
                            acts.append((h[0:m1p, 1, cs], pm1[:]))

                        # m0 (units 0:128): wx (+bias), stale h1, fresh h0 last
                        mm0 = [(ps[:], wxb_sb[:, 0:128], xw[:, r, cs])]
                        if j >= 2:
                            mm0.append((ps[:], *c1(slice(0, 128))))
                        if j >= 1:
                            mm0.append((ps[:], wh_sb[:, 0, 0:128], h[:, 0, cs]))
                        emit_group(mm0)
                        acts.append((h[:, 0, cs], ps[:]))
                    for o, i_ in acts:
                        nc.scalar.activation(o, i_, TANH)

                # output MLP: hid = relu(W1^T h + b1); y = W2 . hid
                hid = mpool.tile([128, 4, BC], DT, tag="hid")
                for m in range(4):
                    ms = slice(m * 128, (m + 1) * 128)
                    pm = ps2pool.tile([128, BC], F32, tag="mlp")
                    nc.tensor.matmul(pm[:], w1_sb[:, 0, ms], h[:, 0, :], start=True, stop=False)
                    nc.tensor.matmul(pm[:], w1_sb[:, 1, ms], h[:, 1, :], start=False, stop=True)
                    nc.scalar.activation(hid[:, m, :], pm[:], RELU, bias=b1_sb[:, m:m + 1])
                yp = ps2pool.tile([1, BC], F32, tag="yp")
                for m in range(4):
                    nc.tensor.matmul(yp[:], w2_sb[:, m:m + 1], hid[:, m, :],
                                     start=(m == 0), stop=(m == 3))
                ysb = mpool.tile([1, BC], F32, tag="ysb")
                nc.vector.tensor_copy(ysb[:], yp[:])
                nc.sync.dma_start(y_d[:], ysb[:])

    nc.compile()
    return nc


def make_in_maps(x, w_x, w_h, b, W1, b1, W2, mm_dt=MM_DT):
    feat = _feat_mask()
    xm = np.asarray(x, np.float32) * feat[None, :, :]   # [B, T, DX]
    xt = np.empty((KX, T, B), np.float32)
    xt[:DX] = xm.transpose(2, 1, 0)
    xt[DX] = 1.0
    xt = _host_cast(xt, mm_dt)
    in_maps = []
    for core in range(8):
        h_idx, s = divmod(core, 2)
        wxb = np.concatenate([np.asarray(w_x[h_idx], np.float32),
                              np.asarray(b[h_idx], np.float32)[None, :]], axis=0)
        in_maps.append({
            "xt": np.ascontiguousarray(xt[:, :, s * BC:(s + 1) * BC]),
            "wh": _host_cast(w_h[h_idx], mm_dt),
            "wxb": _host_cast(wxb, mm_dt),
            "w1": _host_cast(W1[h_idx], mm_dt),
            "b1t": np.ascontiguousarray(np.asarray(b1[h_idx], np.float32).reshape(4, 128).T),
            "w2t": _host_cast(np.asarray(W2[h_idx], np.float32).reshape(4, 128).T, mm_dt),
        })
    return in_maps


def kernel(x, w_x, w_h, b, W1, b1, W2, b2):
    key = (MM_DT, NCH.get(MM_DT), MERGE3)
    if key not in _nc_cache:
        _nc_cache[key] = build_nc()
    nc = _nc_cache[key]
    in_maps = make_in_maps(x, w_x, w_h, b, W1, b1, W2)
    res = run_bass_kernel_spmd(nc, in_maps, core_ids=list(range(8)))
    b2 = np.asarray(b2, np.float32)
    y = np.empty((B, DY), np.float32)
    for core in range(8):
        h_idx, s = divmod(core, 2)
        y[s * BC:(s + 1) * BC, h_idx] = res.results[core]["y"][0] + b2[h_idx]
    return y



# revision 2
# speedup vs baseline: 1.2200x; 1.2200x over previous
"""Clockwork RNN (CWRNN) Trainium2 Bass kernel, v2.

Problem (hardcoded): B=512, T=192, DX=32, DY=4 heads, DH1=256, DH2=512,
update rates (1,2,4) over hidden blocks of (128,64,64) units.

Sharding: 8 cores = 4 heads x 2 batch-halves (B_core=256).

The kernel is latency-bound: 192 serial rounds of (matmul -> tanh) for the
h0 block (units 0:128, updated every step). v2 structures everything
around that chain:
  - ScalarE (ACT) runs ONLY the h0 tanh (one act [128,256] per step).
  - The h1/h2 tanh (updated every 2nd/4th step) runs on the otherwise-idle
    VectorE as a clamped odd polynomial (deg-7 minimax on |z|<=1.8; the
    true preactivation range of this problem is |z| < 1.6).
  - PE order per step puts the matmul whose dependency resolves LAST as
    the accumulation-group stop: type-1 steps end on c1 (needs fresh
    h1/h2 from j-1); type-2/3 steps end on h0 (needs fresh h0 from j-1).
  - x windows are DMA-prefetched one 16-step window ahead.
"""

import contextlib

import numpy as np
import ml_dtypes

import concourse.bass as bass
import concourse.mybir as mybir
import concourse.tile as tile
from concourse import bacc
from concourse.bass_utils import run_bass_kernel_spmd

F32 = mybir.dt.float32
TANH = mybir.ActivationFunctionType.Tanh
RELU = mybir.ActivationFunctionType.Relu
ALU = mybir.AluOpType

B, T, DX, DY, DH1, DH2 = 512, 192, 32, 4, 256, 512
KX = DX + 1          # w_x rows + folded bias row
BC = B // 2          # batch per core (256)
TW = 16              # timesteps per x window DMA

# h12 tanh engine: "dve" (poly on VectorE, keeps ACT exclusive to the h0
# chain) or "act" (tanh on ScalarE like the h0 block)
H12 = "act"
# for H12=="act": merge type-3 m0+m1 into one activation
MERGE3 = True
# clamped odd minimax polys for tanh on |z|<=1.8 (true range |z|<1.6)
CLAMP = 1.8
POLY = {
    2: [0.9686509182659033, -0.2290561478353191, 0.02890260780786102],
    3: [0.9919153997316025, -0.2914799006058794, 0.06928380242107122,
        -0.007339671870479415],
}
DEG = 3

_nc_cache = {}


def _null():
    return contextlib.nullcontext()


def _mm_dtype(name):
    return {"f32r": mybir.dt.float32r, "bf16": mybir.dt.bfloat16,
            "f16": mybir.dt.float16, "f32": mybir.dt.float32}[name]


def _round_f32r(a):
    v = np.ascontiguousarray(a, np.float32).view(np.uint32)
    r = ((v + np.uint32(0x800)) & np.uint32(0xFFFFF000))
    return r.view(np.float32).copy()


def _host_cast(a, dt_name):
    if dt_name == "f32r":
        return _round_f32r(a)
    if dt_name == "bf16":
        return np.asarray(a, np.float32).astype(ml_dtypes.bfloat16)
    if dt_name == "f16":
        return np.asarray(a, np.float32).astype(np.float16)
    return np.ascontiguousarray(a, np.float32)


def _step_type(j):
    if (j + 1) % 4 == 0:
        return 3
    if (j + 1) % 2 == 0:
        return 2
    return 1


def _feat_mask():
    feat = np.zeros((T, DX), np.float32)
    for j in range(T):
        n = {1: 16, 2: 24, 3: 32}[_step_type(j)]
        feat[j, :n] = 1.0
    return feat


def build_nc(mm_dt="f16", h12=None, merge3=None, reps=1, deg=None):
    DT = _mm_dtype(mm_dt)
    if h12 is None:
        h12 = H12
    if merge3 is None:
        merge3 = MERGE3
    if deg is None:
        deg = DEG
    coef = POLY[deg]
    nc = bacc.Bacc("TRN2", target_bir_lowering=False, debug=False)
    xt_d = nc.dram_tensor("xt", [KX, T, BC], DT, kind="ExternalInput")
    wh_d = nc.dram_tensor("wh", [DH1, DH1], DT, kind="ExternalInput")
    wxb_d = nc.dram_tensor("wxb", [KX, DH1], DT, kind="ExternalInput")
    w1_d = nc.dram_tensor("w1", [DH1, DH2], DT, kind="ExternalInput")
    b1_d = nc.dram_tensor("b1t", [128, 4], F32, kind="ExternalInput")
    w2_d = nc.dram_tensor("w2t", [128, 4], DT, kind="ExternalInput")
    y_d = nc.dram_tensor("y", [1, BC], F32, kind="ExternalOutput")

    nwin = T // TW

    with tile.TileContext(nc) as tc:
        with (
            tc.tile_pool(name="const", bufs=1) as cpool,
            tc.tile_pool(name="state", bufs=1) as spool,
            tc.tile_pool(name="xw", bufs=2) as xpool,
            tc.tile_pool(name="pm0", bufs=2, space=bass.MemorySpace.PSUM) as pm0pool,
            tc.tile_pool(name="pm1", bufs=2, space=bass.MemorySpace.PSUM) as pm1pool,
            tc.tile_pool(name="ps2", bufs=1, space=bass.MemorySpace.PSUM) as ps2pool,
            tc.tile_pool(name="poly", bufs=1) as ppool,
            tc.tile_pool(name="misc", bufs=1) as mpool,
        ):
            wh_sb = cpool.tile([128, 2, DH1], DT, tag="wh")
            for c in range(2):
                nc.sync.dma_start(wh_sb[:, c, :], wh_d[c * 128:(c + 1) * 128, :])
            wxb_sb = cpool.tile([KX, DH1], DT, tag="wxb")
            nc.sync.dma_start(wxb_sb[:], wxb_d[:])
            w1_sb = cpool.tile([128, 2, DH2], DT, tag="w1")
            for c in range(2):
                nc.sync.dma_start(w1_sb[:, c, :], w1_d[c * 128:(c + 1) * 128, :])
            b1_sb = cpool.tile([128, 4], F32, tag="b1")
            nc.sync.dma_start(b1_sb[:], b1_d[:])
            w2_sb = cpool.tile([128, 4], DT, tag="w2")
            nc.sync.dma_start(w2_sb[:], w2_d[:])

            # h[:, 0, :] = units 0:128 (h0); h[:, 1, :] = units 128:256.
            # Never zero-initialized: matmuls on not-yet-written blocks are
            # skipped (j=0,1) or K-narrowed (j<=3) instead.
            h = spool.tile([128, 2, BC], DT, tag="h")
            # poly intermediates (f16 sbuf so DVE runs in 2x mode)
            zt = ppool.tile([128, BC], DT, tag="zt")
            tt = ppool.tile([128, BC], DT, tag="tt")
            pt = ppool.tile([128, BC], DT, tag="pt")

            def dve_tanh(out, psum_in, m1u):
                # out = tanh(psum_in) via clamped odd poly, entirely on DVE
                z, t, p = zt[0:m1u, :], tt[0:m1u, :], pt[0:m1u, :]
                nc.vector.tensor_scalar(out=z, in0=psum_in, scalar1=-CLAMP,
                                        scalar2=CLAMP, op0=ALU.max, op1=ALU.min)
                nc.vector.tensor_tensor(out=t, in0=z, in1=z, op=ALU.mult)
                nc.vector.tensor_scalar(out=p, in0=t, scalar1=coef[deg],
                                        scalar2=coef[deg - 1], op0=ALU.mult,
                                        op1=ALU.add)
                for k in range(deg - 2, -1, -1):
                    nc.vector.tensor_tensor(out=p, in0=p, in1=t, op=ALU.mult)
                    nc.vector.tensor_scalar(out=p, in0=p, scalar1=coef[k],
                                            scalar2=None, op0=ALU.add)
                nc.vector.tensor_tensor(out=out, in0=p, in1=z, op=ALU.mult)

            with tc.For_i(0, reps, 1) if reps > 1 else _null():
                xw = [None, None]
                for j in range(T):
                    r = j % TW
                    w = j // TW
                    if j == 0:
                        xw[0] = xpool.tile([KX, TW, BC], DT, tag="xw0",
                                           name="xw_a")
                        nc.sync.dma_start(xw[0][:], xt_d[:, 0:TW, :])
                    if r == 0 and w + 1 < nwin:
                        nxt = xpool.tile([KX, TW, BC], DT, tag=f"xw{(w + 1) % 2}",
                                         name="xw_b")
                        nc.sync.dma_start(nxt[:], xt_d[:, (w + 1) * TW:(w + 2) * TW, :])
                        xw[(w + 1) % 2] = nxt
                    xcur = xw[w % 2]

                    typ = _step_type(j)
                    m1u = {1: 0, 2: 64, 3: 128}[typ]
                    k1 = 64 if j <= 3 else 128  # h2 first written at j=3

                    merged = typ == 3 and merge3 and h12 == "act"
                    if merged:
                        pm = pm0pool.tile([128, 2, BC], F32, tag="pm0", name="pmM")
                        pm0 = pm[:, 0, :]
                        pm1 = pm[:, 1, :]
                    else:
                        pm0 = pm0pool.tile([128, BC], F32, tag="pm0", name="pm0")[:]
                        pm1 = None
                        if m1u:
                            pm1 = pm1pool.tile([128, BC], F32, tag="pm1",
                                               name="pm1")[0:m1u, :]

                    # ---- matmuls ----
                    # entries: (group_id, name, psum_slice, lhsT, rhs)
                    mms = [(0, "x", pm0, wxb_sb[:, 0:128], xcur[:, r, :])]
                    if m1u:
                        mms.append((1, "x", pm1, wxb_sb[:, 128:128 + m1u],
                                    xcur[:, r, :]))
                    if j >= 2:
                        mms.append((0, "c1", pm0, wh_sb[0:k1, 1, 0:128],
                                    h[0:k1, 1, :]))
                        if m1u:
                            mms.append((1, "c1", pm1,
                                        wh_sb[0:k1, 1, 128:128 + m1u],
                                        h[0:k1, 1, :]))
                    if j >= 1:
                        mms.append((0, "h0", pm0, wh_sb[:, 0, 0:128], h[:, 0, :]))
                        if m1u:
                            mms.append((1, "h0", pm1, wh_sb[:, 0, 128:128 + m1u],
                                        h[:, 0, :]))

                    # emission order: leads (inputs not written by the
                    # previous step's acts) first, then act-gated tails with
                    # the act0-gating m0 matmul absolutely last. g0/g1 are
                    # separate psum tiles, so group interleaving is safe.
                    def gated(nm):
                        return nm == "h0" or (nm == "c1" and typ == 1 and j >= 2)

                    order = ([t for t in mms if not gated(t[1])]
                             + [t for t in mms if gated(t[1]) and t[0] == 1]
                             + [t for t in mms if gated(t[1]) and t[0] == 0])
                    if merged:
                        # one tile: single accumulation group across both
                        # regions (start's bank-wide has_written clear means
                        # interleaved per-region groups would corrupt)
                        for k, (gi, nm, o, lt, rh) in enumerate(order):
                            nc.tensor.matmul(o, lt, rh, start=(k == 0),
                                             stop=(k == len(order) - 1),
                                             skip_group_check=True)
                    else:
                        cnt = {0: sum(1 for t in mms if t[0] == 0),
                               1: sum(1 for t in mms if t[0] == 1)}
                        seen = {0: 0, 1: 0}
                        for gi, nm, o, lt, rh in order:
                            nc.tensor.matmul(o, lt, rh, start=(seen[gi] == 0),
                                             stop=(seen[gi] == cnt[gi] - 1))
                            seen[gi] += 1

                    # ---- activations ----
                    if merged:
                        nc.scalar.activation(h[:, 0:2, :], pm[:], TANH)
                    else:
                        nc.scalar.activation(h[:, 0, :], pm0, TANH)
                        if m1u:
                            if h12 == "dve":
                                dve_tanh(h[0:m1u, 1, :], pm1, m1u)
                            else:
                                nc.scalar.activation(h[0:m1u, 1, :], pm1, TANH)

                # ---- output MLP: hid = relu(W1^T h + b1); y = W2 . hid ----
                hid = mpool.tile([128, 4, BC], DT, tag="hid")
                for m in range(4):
                    ms = slice(m * 128, (m + 1) * 128)
                    pmlp = ps2pool.tile([128, BC], F32, tag="mlp", name="pmlp")
                    nc.tensor.matmul(pmlp[:], w1_sb[:, 0, ms], h[:, 0, :],
                                     start=True, stop=False)
                    nc.tensor.matmul(pmlp[:], w1_sb[:, 1, ms], h[:, 1, :],
                                     start=False, stop=True)
                    nc.scalar.activation(hid[:, m, :], pmlp[:], RELU,
                                         bias=b1_sb[:, m:m + 1])
                yp = ps2pool.tile([1, BC], F32, tag="yp", name="yp")
                for m in range(4):
                    nc.tensor.matmul(yp[:], w2_sb[:, m:m + 1], hid[:, m, :],
                                     start=(m == 0), stop=(m == 3))
                ysb = mpool.tile([1, BC], F32, tag="ysb")
                nc.vector.tensor_copy(ysb[:], yp[:])
                nc.sync.dma_start(y_d[:], ysb[:])

    nc.compile()
    return nc


def make_in_maps(x, w_x, w_h, b, W1, b1, W2, mm_dt="f16"):
    feat = _feat_mask()
    xm = np.asarray(x, np.float32) * feat[None, :, :]   # [B, T, DX]
    xt = np.empty((KX, T, B), np.float32)
    xt[:DX] = xm.transpose(2, 1, 0)
    xt[DX] = 1.0
    xt = _host_cast(xt, mm_dt)
    in_maps = []
    for core in range(8):
        h_idx, s = divmod(core, 2)
        wxb = np.concatenate([np.asarray(w_x[h_idx], np.float32),
                              np.asarray(b[h_idx], np.float32)[None, :]], axis=0)
        in_maps.append({
            "xt": np.ascontiguousarray(xt[:, :, s * BC:(s + 1) * BC]),
            "wh": _host_cast(w_h[h_idx], mm_dt),
            "wxb": _host_cast(wxb, mm_dt),
            "w1": _host_cast(W1[h_idx], mm_dt),
            "b1t": np.ascontiguousarray(np.asarray(b1[h_idx], np.float32).reshape(4, 128).T),
            "w2t": _host_cast(np.asarray(W2[h_idx], np.float32).reshape(4, 128).T, mm_dt),
        })
    return in_maps


MM_DT = "f16"


def kernel(x, w_x, w_h, b, W1, b1, W2, b2):
    key = (MM_DT, H12, MERGE3, DEG)
    if key not in _nc_cache:
        _nc_cache[key] = build_nc(MM_DT)
    nc = _nc_cache[key]
    in_maps = make_in_maps(x, w_x, w_h, b, W1, b1, W2, mm_dt=MM_DT)
    res = run_bass_kernel_spmd(nc, in_maps, core_ids=list(range(8)))
    b2 = np.asarray(b2, np.float32)
    y = np.empty((B, DY), np.float32)
    for core in range(8):
        h_idx, s = divmod(core, 2)
        y[s * BC:(s + 1) * BC, h_idx] = res.results[core]["y"][0] + b2[h_idx]
    return y


# revision 3
# speedup vs baseline: 1.2243x; 1.0035x over previous
"""Clockwork RNN (CWRNN) Trainium2 Bass kernel, v2.

Problem (hardcoded): B=512, T=192, DX=32, DY=4 heads, DH1=256, DH2=512,
update rates (1,2,4) over hidden blocks of (128,64,64) units.

Sharding: 8 cores = 4 heads x 2 batch-halves (B_core=256).

The kernel is latency-bound: 192 serial rounds of (matmul -> tanh) for the
h0 block (units 0:128, updated every step). v2 structures everything
around that chain:
  - ScalarE (ACT) runs ONLY the h0 tanh (one act [128,256] per step).
  - The h1/h2 tanh (updated every 2nd/4th step) runs on the otherwise-idle
    VectorE as a clamped odd polynomial (deg-7 minimax on |z|<=1.8; the
    true preactivation range of this problem is |z| < 1.6).
  - PE order per step puts the matmul whose dependency resolves LAST as
    the accumulation-group stop: type-1 steps end on c1 (needs fresh
    h1/h2 from j-1); type-2/3 steps end on h0 (needs fresh h0 from j-1).
  - x windows are DMA-prefetched one 16-step window ahead.
"""

import contextlib

import numpy as np
import ml_dtypes

import concourse.bass as bass
import concourse.mybir as mybir
import concourse.tile as tile
from concourse import bacc
from concourse.bass_utils import run_bass_kernel_spmd

F32 = mybir.dt.float32
TANH = mybir.ActivationFunctionType.Tanh
RELU = mybir.ActivationFunctionType.Relu
ALU = mybir.AluOpType

B, T, DX, DY, DH1, DH2 = 512, 192, 32, 4, 256, 512
KX = DX + 1          # w_x rows + folded bias row
BC = B // 2          # batch per core (256)
TW = 16              # timesteps per x window DMA

# h12 tanh engine: "dve" (poly on VectorE, keeps ACT exclusive to the h0
# chain) or "act" (tanh on ScalarE like the h0 block)
H12 = "act"
# for H12=="act": merge type-3 m0+m1 into one activation
MERGE3 = True
# clamped odd minimax polys for tanh on |z|<=1.8 (true range |z|<1.6)
CLAMP = 1.8
POLY = {
    2: [0.9686509182659033, -0.2290561478353191, 0.02890260780786102],
    3: [0.9919153997316025, -0.2914799006058794, 0.06928380242107122,
        -0.007339671870479415],
}
DEG = 3

_nc_cache = {}


def _null():
    return contextlib.nullcontext()


def _mm_dtype(name):
    return {"f32r": mybir.dt.float32r, "bf16": mybir.dt.bfloat16,
            "f16": mybir.dt.float16, "f32": mybir.dt.float32}[name]


def _round_f32r(a):
    v = np.ascontiguousarray(a, np.float32).view(np.uint32)
    r = ((v + np.uint32(0x800)) & np.uint32(0xFFFFF000))
    return r.view(np.float32).copy()


def _host_cast(a, dt_name):
    if dt_name == "f32r":
        return _round_f32r(a)
    if dt_name == "bf16":
        return np.asarray(a, np.float32).astype(ml_dtypes.bfloat16)
    if dt_name == "f16":
        return np.asarray(a, np.float32).astype(np.float16)
    return np.ascontiguousarray(a, np.float32)


def _step_type(j):
    if (j + 1) % 4 == 0:
        return 3
    if (j + 1) % 2 == 0:
        return 2
    return 1


def _feat_mask():
    feat = np.zeros((T, DX), np.float32)
    for j in range(T):
        n = {1: 16, 2: 24, 3: 32}[_step_type(j)]
        feat[j, :n] = 1.0
    return feat


def build_nc(mm_dt="f16", h12=None, merge3=None, reps=1, deg=None):
    DT = _mm_dtype(mm_dt)
    if h12 is None:
        h12 = H12
    if merge3 is None:
        merge3 = MERGE3
    if deg is None:
        deg = DEG
    coef = POLY[deg]
    nc = bacc.Bacc("TRN2", target_bir_lowering=False, debug=False)
    xt_d = nc.dram_tensor("xt", [KX, T, BC], DT, kind="ExternalInput")
    wh_d = nc.dram_tensor("wh", [DH1, DH1], DT, kind="ExternalInput")
    wxb_d = nc.dram_tensor("wxb", [KX, DH1], DT, kind="ExternalInput")
    w1_d = nc.dram_tensor("w1", [DH1, DH2], DT, kind="ExternalInput")
    b1_d = nc.dram_tensor("b1t", [128, 4], F32, kind="ExternalInput")
    w2_d = nc.dram_tensor("w2t", [128, 4], DT, kind="ExternalInput")
    y_d = nc.dram_tensor("y", [1, BC], F32, kind="ExternalOutput")

    nwin = T // TW

    with tile.TileContext(nc) as tc:
        with (
            tc.tile_pool(name="const", bufs=1) as cpool,
            tc.tile_pool(name="state", bufs=1) as spool,
            tc.tile_pool(name="xw", bufs=2) as xpool,
            tc.tile_pool(name="pm0", bufs=2, space=bass.MemorySpace.PSUM) as pm0pool,
            tc.tile_pool(name="pm1", bufs=2, space=bass.MemorySpace.PSUM) as pm1pool,
            tc.tile_pool(name="ps2", bufs=2, space=bass.MemorySpace.PSUM) as ps2pool,
            tc.tile_pool(name="poly", bufs=1) as ppool,
            tc.tile_pool(name="misc", bufs=1) as mpool,
        ):
            wh_sb = cpool.tile([128, 2, DH1], DT, tag="wh")
            for c in range(2):
                nc.sync.dma_start(wh_sb[:, c, :], wh_d[c * 128:(c + 1) * 128, :])
            wxb_sb = cpool.tile([KX, DH1], DT, tag="wxb")
            nc.sync.dma_start(wxb_sb[:], wxb_d[:])
            w1_sb = cpool.tile([128, 2, DH2], DT, tag="w1")
            for c in range(2):
                nc.sync.dma_start(w1_sb[:, c, :], w1_d[c * 128:(c + 1) * 128, :])
            b1_sb = cpool.tile([128, 4], F32, tag="b1")
            nc.sync.dma_start(b1_sb[:], b1_d[:])
            w2_sb = cpool.tile([128, 4], DT, tag="w2")
            nc.sync.dma_start(w2_sb[:], w2_d[:])

            # h[:, 0, :] = units 0:128 (h0); h[:, 1, :] = units 128:256.
            # Never zero-initialized: matmuls on not-yet-written blocks are
            # skipped (j=0,1) or K-narrowed (j<=3) instead.
            h = spool.tile([128, 2, BC], DT, tag="h")
            # poly intermediates (f16 sbuf so DVE runs in 2x mode)
            zt = ppool.tile([128, BC], DT, tag="zt")
            tt = ppool.tile([128, BC], DT, tag="tt")
            pt = ppool.tile([128, BC], DT, tag="pt")

            def dve_tanh(out, psum_in, m1u):
                # out = tanh(psum_in) via clamped odd poly, entirely on DVE
                z, t, p = zt[0:m1u, :], tt[0:m1u, :], pt[0:m1u, :]
                nc.vector.tensor_scalar(out=z, in0=psum_in, scalar1=-CLAMP,
                                        scalar2=CLAMP, op0=ALU.max, op1=ALU.min)
                nc.vector.tensor_tensor(out=t, in0=z, in1=z, op=ALU.mult)
                nc.vector.tensor_scalar(out=p, in0=t, scalar1=coef[deg],
                                        scalar2=coef[deg - 1], op0=ALU.mult,
                                        op1=ALU.add)
                for k in range(deg - 2, -1, -1):
                    nc.vector.tensor_tensor(out=p, in0=p, in1=t, op=ALU.mult)
                    nc.vector.tensor_scalar(out=p, in0=p, scalar1=coef[k],
                                            scalar2=None, op0=ALU.add)
                nc.vector.tensor_tensor(out=out, in0=p, in1=z, op=ALU.mult)

            with tc.For_i(0, reps, 1) if reps > 1 else _null():
                xw = [None, None]
                for j in range(T):
                    r = j % TW
                    w = j // TW
                    if j == 0:
                        xw[0] = xpool.tile([KX, TW, BC], DT, tag="xw0",
                                           name="xw_a")
                        nc.sync.dma_start(xw[0][:], xt_d[:, 0:TW, :])
                    if r == 0 and w + 1 < nwin:
                        nxt = xpool.tile([KX, TW, BC], DT, tag=f"xw{(w + 1) % 2}",
                                         name="xw_b")
                        nc.sync.dma_start(nxt[:], xt_d[:, (w + 1) * TW:(w + 2) * TW, :])
                        xw[(w + 1) % 2] = nxt
                    xcur = xw[w % 2]

                    typ = _step_type(j)
                    m1u = {1: 0, 2: 64, 3: 128}[typ]
                    k1 = 64 if j <= 3 else 128  # h2 first written at j=3

                    merged = typ == 3 and merge3 and h12 == "act"
                    if merged:
                        pm = pm0pool.tile([128, 2, BC], F32, tag="pm0", name="pmM")
                        pm0 = pm[:, 0, :]
                        pm1 = pm[:, 1, :]
                    else:
                        pm0 = pm0pool.tile([128, BC], F32, tag="pm0", name="pm0")[:]
                        pm1 = None
                        if m1u:
                            pm1 = pm1pool.tile([128, BC], F32, tag="pm1",
                                               name="pm1")[0:m1u, :]

                    # ---- matmuls ----
                    # entries: (group_id, name, psum_slice, lhsT, rhs)
                    mms = [(0, "x", pm0, wxb_sb[:, 0:128], xcur[:, r, :])]
                    if m1u:
                        mms.append((1, "x", pm1, wxb_sb[:, 128:128 + m1u],
                                    xcur[:, r, :]))
                    if j >= 2:
                        mms.append((0, "c1", pm0, wh_sb[0:k1, 1, 0:128],
                                    h[0:k1, 1, :]))
                        if m1u:
                            mms.append((1, "c1", pm1,
                                        wh_sb[0:k1, 1, 128:128 + m1u],
                                        h[0:k1, 1, :]))
                    if j >= 1:
                        mms.append((0, "h0", pm0, wh_sb[:, 0, 0:128], h[:, 0, :]))
                        if m1u:
                            mms.append((1, "h0", pm1, wh_sb[:, 0, 128:128 + m1u],
                                        h[:, 0, :]))

                    # emission order: leads (inputs not written by the
                    # previous step's acts) first, then act-gated tails with
                    # the act0-gating m0 matmul absolutely last. g0/g1 are
                    # separate psum tiles, so group interleaving is safe.
                    def gated(nm):
                        return nm == "h0" or (nm == "c1" and typ == 1 and j >= 2)

                    order = ([t for t in mms if not gated(t[1])]
                             + [t for t in mms if gated(t[1]) and t[0] == 1]
                             + [t for t in mms if gated(t[1]) and t[0] == 0])
                    if merged:
                        # one tile: single accumulation group across both
                        # regions (start's bank-wide has_written clear means
                        # interleaved per-region groups would corrupt)
                        for k, (gi, nm, o, lt, rh) in enumerate(order):
                            nc.tensor.matmul(o, lt, rh, start=(k == 0),
                                             stop=(k == len(order) - 1),
                                             skip_group_check=True)
                    else:
                        cnt = {0: sum(1 for t in mms if t[0] == 0),
                               1: sum(1 for t in mms if t[0] == 1)}
                        seen = {0: 0, 1: 0}
                        for gi, nm, o, lt, rh in order:
                            nc.tensor.matmul(o, lt, rh, start=(seen[gi] == 0),
                                             stop=(seen[gi] == cnt[gi] - 1))
                            seen[gi] += 1

                    # ---- activations ----
                    if merged:
                        nc.scalar.activation(h[:, 0:2, :], pm[:], TANH)
                    else:
                        nc.scalar.activation(h[:, 0, :], pm0, TANH)
                        if m1u:
                            if h12 == "dve":
                                dve_tanh(h[0:m1u, 1, :], pm1, m1u)
                            else:
                                nc.scalar.activation(h[0:m1u, 1, :], pm1, TANH)

                # ---- output MLP: hid = relu(W1^T h + b1); y = W2 . hid ----
                hid = mpool.tile([128, 4, BC], DT, tag="hid")
                for m in range(4):
                    ms = slice(m * 128, (m + 1) * 128)
                    pmlp = ps2pool.tile([128, BC], F32, tag="mlp", name="pmlp")
                    nc.tensor.matmul(pmlp[:], w1_sb[:, 0, ms], h[:, 0, :],
                                     start=True, stop=False)
                    nc.tensor.matmul(pmlp[:], w1_sb[:, 1, ms], h[:, 1, :],
                                     start=False, stop=True)
                    nc.scalar.activation(hid[:, m, :], pmlp[:], RELU,
                                         bias=b1_sb[:, m:m + 1])
                yp = ps2pool.tile([1, BC], F32, tag="yp", name="yp")
                for m in range(4):
                    nc.tensor.matmul(yp[:], w2_sb[:, m:m + 1], hid[:, m, :],
                                     start=(m == 0), stop=(m == 3))
                ysb = mpool.tile([1, BC], F32, tag="ysb")
                nc.vector.tensor_copy(ysb[:], yp[:])
                nc.sync.dma_start(y_d[:], ysb[:])

    nc.compile()
    return nc


def make_in_maps(x, w_x, w_h, b, W1, b1, W2, mm_dt="f16"):
    feat = _feat_mask()
    xm = np.asarray(x, np.float32) * feat[None, :, :]   # [B, T, DX]
    xt = np.empty((KX, T, B), np.float32)
    xt[:DX] = xm.transpose(2, 1, 0)
    xt[DX] = 1.0
    xt = _host_cast(xt, mm_dt)
    in_maps = []
    for core in range(8):
        h_idx, s = divmod(core, 2)
        wxb = np.concatenate([np.asarray(w_x[h_idx], np.float32),
                              np.asarray(b[h_idx], np.float32)[None, :]], axis=0)
        in_maps.append({
            "xt": np.ascontiguousarray(xt[:, :, s * BC:(s + 1) * BC]),
            "wh": _host_cast(w_h[h_idx], mm_dt),
            "wxb": _host_cast(wxb, mm_dt),
            "w1": _host_cast(W1[h_idx], mm_dt),
            "b1t": np.ascontiguousarray(np.asarray(b1[h_idx], np.float32).reshape(4, 128).T),
            "w2t": _host_cast(np.asarray(W2[h_idx], np.float32).reshape(4, 128).T, mm_dt),
        })
    return in_maps


MM_DT = "f16"


def kernel(x, w_x, w_h, b, W1, b1, W2, b2):
    key = (MM_DT, H12, MERGE3, DEG)
    if key not in _nc_cache:
        _nc_cache[key] = build_nc(MM_DT)
    nc = _nc_cache[key]
    in_maps = make_in_maps(x, w_x, w_h, b, W1, b1, W2, mm_dt=MM_DT)
    res = run_bass_kernel_spmd(nc, in_maps, core_ids=list(range(8)))
    b2 = np.asarray(b2, np.float32)
    y = np.empty((B, DY), np.float32)
    for core in range(8):
        h_idx, s = divmod(core, 2)
        y[s * BC:(s + 1) * BC, h_idx] = res.results[core]["y"][0] + b2[h_idx]
    return y
